# revision 40
# baseline (speedup 1.0000x reference)
"""GCN (3x GCNConv + global mean pool + linear) on 8 Trainium2 NeuronCores.

Strategy (dst-sharded message passing, v2):
  - Nodes sharded n/8 per core; each core's nodes permuted into windows of
    32 (degree-balanced) -> supertiles of 128 (PSUM tiles).
  - Edges partitioned by dst core and packed into (window, class) tiles of
    128 edges; class = which signed-int16-indexable half of the node table
    the src row lives in (dma_gather indices are int16).
  - Normalization dinv = rsqrt(deg+1) is computed on host and folded into
    host-built scatter tiles S [128e, 32d] fp16 (coef = dinv_src*ew*dinv_dst),
    kept SBUF-resident for all three layers. Tables store PLAIN h (fp16).
  - Per layer: dma_gather pulls 256B fp16 rows of the node table from HBM,
    rotating over SWDGE queues 1..3 so three Q7 core-pairs generate DMA
    descriptors concurrently (queue 0 gathers run synchronously on the Pool
    engine; 1..3 retire early and overlap); TensorE computes the
    scatter-add as S^T @ M matmuls accumulated per window in PSUM.
  - Per supertile: u = agg + hown2 (hown2 = h*dinv^2, resident in SBUF),
    PE transpose, f32 GEMM with W, bias(+relu) on ACT, transpose back,
    fp16 table write. Tables distributed with AllGather.
  - Pooling: matmul with host-built P (1[batch==g]) accumulated over
    supertiles -> AllReduce -> final linear on-device -> out [64, 5] f32.
"""

import os
import sys
import numpy as np

for _p in ("/opt/trn_rl_repo", "/root/.axon_site/_ro/trn_rl_repo"):
    if os.path.isdir(_p) and _p not in sys.path:
        sys.path.insert(0, _p)

N_CORES = 8
N_GRAPHS = 64
HID = 128
N_CLASS = 5
F_IN = 7
F_PAD = 8
WIN = 32
SUP = 128
GROUP_SUPS = 3
CHUNK_MAX = 24
GATH_BUFS = 8
SINGLE_PACKET = False
IDX_CAP = 32768
GATHER_QUEUES = (1, 2, 3, 0)
NSA = 24  # supertiles in the early-AllGather half (multiple of GROUP_SUPS)


def _group_ranks(keys, n_keys):
    """rank of each element within its key group (keys int array)."""
    nk = len(keys)
    if nk == 0:
        return np.zeros(0, dtype=np.int64)
    order = np.argsort(keys, kind="stable")
    sk = keys[order]
    is_new = np.r_[True, sk[1:] != sk[:-1]]
    gs_idx = np.nonzero(is_new)[0]
    gs = np.repeat(gs_idx, np.diff(np.r_[gs_idx, nk]))
    rank = np.empty(nk, dtype=np.int64)
    rank[order] = np.arange(nk) - gs
    return rank


class Plan:
    pass


def build_plan(x, edge_index, edge_attr, batch, n_cores=N_CORES, n_graphs=N_GRAPHS):
    """Host-side sharding/layout planning + normalization coefficients."""
    p = Plan()
    n = x.shape[0]
    assert n % n_cores == 0
    npc = n // n_cores
    nsup = (npc + SUP - 1) // SUP
    padc = nsup * SUP
    npad = n_cores * padc
    nwin = padc // WIN
    p.n, p.npc, p.nsup, p.padc, p.npad, p.nwin = n, npc, nsup, padc, npad, nwin
    p.n_cores, p.n_graphs = n_cores, n_graphs
    base_hi = max(0, npad - IDX_CAP)
    p.base_hi = base_hi

    src = np.asarray(edge_index[0], dtype=np.int64)
    dst = np.asarray(edge_index[1], dtype=np.int64)
    ew = np.asarray(edge_attr, dtype=np.float32)
    batch = np.asarray(batch, dtype=np.int64)

    # ---- normalization (host): deg = sum of incoming ew + 1 (self loop) ----
    deg = np.bincount(dst, weights=ew.astype(np.float64), minlength=n) + 1.0
    dinv = (1.0 / np.sqrt(deg)).astype(np.float32)
    p.dinv = dinv

    # ---- window assignment: weighted-target greedy balance of edge counts --
    # Target <=512 edges (4 tiles) per window; K coordinated overflow windows
    # absorb each core's excess so the cross-core max stays at the floor.
    import heapq

    indeg = np.bincount(dst, minlength=n)
    core_tot = np.bincount(dst // npc, minlength=n_cores)
    K = int(max(0, (np.max(core_tot) - nwin * 4 * 128 + 127) // 128))
    base = np.full(nwin, 4 * 128.0)
    base[:K] += 128.0
    prow = np.empty(n, dtype=np.int64)
    win_all = np.empty(n, dtype=np.int64)
    for c in range(n_cores):
        lo = c * npc
        deg_c = np.zeros(padc, dtype=np.int64)
        deg_c[:npc] = indeg[lo : lo + npc]  # pad ghost nodes with degree 0
        order = np.argsort(-deg_c, kind="stable")
        tgt = base * (core_tot[c] / base.sum())
        load = np.zeros(nwin)
        wsel = np.empty(padc, dtype=np.int64)
        rnd = np.empty(padc, dtype=np.int64)
        for r in range(WIN):
            block = order[r * nwin : (r + 1) * nwin]  # degrees descending
            worder = np.argsort(load / tgt, kind="stable")  # lightest first
            wsel[block] = worder
            rnd[block] = r
            load[worder] += deg_c[block]
        # repair: node swaps until every window is under its hard cap
        cap = np.full(nwin, 4 * 128.0)
        cap[:K] += 128.0
        for _ in range(2000):
            over = np.nonzero(load > cap)[0]
            if len(over) == 0:
                break
            wh = over[np.argmax((load - cap)[over])]
            excess = load[wh] - cap[wh]
            room = cap - load
            wl = int(np.argmax(room))
            ih = np.nonzero(wsel == wh)[0]
            il = np.nonzero(wsel == wl)[0]
            d = deg_c[ih][:, None] - deg_c[il][None, :]
            valid = (d >= excess) & (d <= room[wl])
            if valid.any():
                dv = np.where(valid, d, np.inf)
                j = np.unravel_index(np.argmin(dv), d.shape)
            else:
                dv = np.where(d <= room[wl], d, -np.inf)
                j = np.unravel_index(np.argmax(dv), d.shape)
                if not np.isfinite(dv[j]) or d[j] <= 0:
                    break
            da, db = ih[j[0]], il[j[1]]
            delta = d[j]
            wsel[da], wsel[db] = wl, wh
            rnd[da], rnd[db] = rnd[db], rnd[da]
            load[wh] -= delta
            load[wl] += delta
        w = wsel[:npc]
        plid = (w // 4) * SUP + (w % 4) * WIN + rnd[:npc]
        prow[lo : lo + npc] = c * padc + plid
        win_all[lo : lo + npc] = w
    p.prow = prow
    # table-row layout: a-region (supertiles < NSA, AllGathered early) then
    # b-region, each rank-major within the region
    na, nb = NSA * SUP, padc - NSA * SUP
    p.na, p.nb = na, nb
    plid_all = prow % padc
    core_all = prow // padc
    trow = np.where(
        plid_all < na,
        core_all * na + plid_all,
        n_cores * na + core_all * nb + (plid_all - na),
    )
    p.trow = trow

    # ---- per-core-window tile budgets (uniform across cores) ----
    ecore = dst // npc
    esrc_prow = trow[src]
    can_lo = esrc_prow < IDX_CAP
    can_hi = esrc_prow >= base_hi
    ewin = win_all[dst]

    flo_cw = np.zeros((n_cores, nwin), dtype=np.int64)
    fhi_cw = np.zeros((n_cores, nwin), dtype=np.int64)
    tot_cw = np.zeros((n_cores, nwin), dtype=np.int64)
    np.add.at(tot_cw, (ecore, ewin), 1)
    np.add.at(flo_cw, (ecore[~can_hi], ewin[~can_hi]), 1)
    np.add.at(fhi_cw, (ecore[~can_lo], ewin[~can_lo]), 1)

    t_lo = np.max((flo_cw + 127) // 128, axis=0)
    t_hi = np.max((fhi_cw + 127) // 128, axis=0)
    grow = np.maximum(np.max(tot_cw, axis=0) - (t_lo + t_hi) * 128, 0)
    t_lo = t_lo + (grow + 127) // 128
    t_lo = np.maximum(t_lo, (t_lo + t_hi) == 0)
    p.t_lo, p.t_hi = t_lo, t_hi

    # ---- global tile order: groups of supertiles, class runs within group --
    n_groups = (nsup + GROUP_SUPS - 1) // GROUP_SUPS
    p.n_groups = n_groups
    tiles = []  # (win, cls)
    chunks = []  # (tile_start, ntiles, cls)
    groups = []  # (sup_start, nsups, [chunk idx], (t0, t1))
    for g in range(n_groups):
        s0 = g * GROUP_SUPS
        ns = min(GROUP_SUPS, nsup - s0)
        wlist = range(s0 * 4, (s0 + ns) * 4)
        g_t0 = len(tiles)
        g_chunks = []
        for cls in (0, 1):
            run_t0 = len(tiles)
            for w in wlist:
                tc = int(t_lo[w]) if cls == 0 else int(t_hi[w])
                tiles.extend((w, cls) for _ in range(tc))
            nrun = len(tiles) - run_t0
            t0 = run_t0
            while nrun > 0:
                take = min(CHUNK_MAX, nrun)
                g_chunks.append(len(chunks))
                chunks.append((t0, take, cls))
                t0 += take
                nrun -= take
        groups.append((s0, ns, g_chunks, (g_t0, len(tiles))))
    p.tiles, p.chunks, p.groups = tiles, chunks, groups
    p.tot = len(tiles)

    # queue assignment: greedy least-loaded (by tile count) over queues 1..3
    qload = {q: 0 for q in GATHER_QUEUES}
    p.chunk_queue = []
    for _t0, ntl, _cls in chunks:
        q = min(qload, key=lambda k: qload[k])
        p.chunk_queue.append(q)
        qload[q] += ntl

    # first/last tile of each window (for PSUM start/stop flags)
    tw = np.array([t[0] for t in tiles])
    p.first_of_win = np.zeros(p.tot, dtype=bool)
    p.last_of_win = np.zeros(p.tot, dtype=bool)
    for w in range(nwin):
        ids = np.nonzero(tw == w)[0]
        p.first_of_win[ids.min()] = True
        p.last_of_win[ids.max()] = True
    # first tile index of each (win, cls) run
    t_off = {}
    for t, (w, cls) in enumerate(tiles):
        t_off.setdefault((w, cls), t)

    counts = np.bincount(batch, minlength=n_graphs).astype(np.float32)
    p.counts = counts

    # ---- per-core arrays ----
    norm = dinv[src] * ew * dinv[dst]  # full edge coefficient
    p.per_core = []
    for c in range(n_cores):
        m = ecore == c
        ed = dst[m]
        enorm = norm[m]
        eprow = esrc_prow[m]
        ewin_c = ewin[m]
        eslot = (prow[ed] % SUP) % WIN  # row within window = rnd
        e_can_hi = can_hi[m]
        e_can_lo = can_lo[m]
        ne = len(ed)

        # per-edge class: fill lo up to its target, rest hi
        ecls = np.full(ne, -1, dtype=np.int64)
        ecls[~e_can_hi] = 0
        ecls[~e_can_lo] = 1
        free = ecls == -1
        tot_w = np.bincount(ewin_c, minlength=nwin)
        flo_w = np.bincount(ewin_c[~e_can_hi], minlength=nwin)
        lo_target = np.maximum(flo_w, tot_w - t_hi * 128)
        lo_target = np.minimum(lo_target, t_lo * 128)
        fidx = np.nonzero(free)[0]
        frank = _group_ranks(ewin_c[fidx], nwin)
        to_lo = frank < (lo_target - flo_w)[ewin_c[fidx]]
        ecls[fidx[to_lo]] = 0
        ecls[fidx[~to_lo]] = 1

        # slot position within (win, cls) run
        key = ewin_c * 2 + ecls
        k = _group_ranks(key, nwin * 2)
        t_off_arr = np.zeros((nwin, 2), dtype=np.int64)
        for (wv, cv), tv in t_off.items():
            t_off_arr[wv, cv] = tv
        run0 = t_off_arr[ewin_c, ecls]
        t_of_e = run0 + k // 128
        p_of_e = k % 128

        idx_arr = np.zeros((p.tot, 128), dtype=np.int16)
        s_arr = np.zeros((p.tot, 128, WIN), dtype=np.float16)
        rel = eprow - np.where(ecls == 1, base_hi, 0)
        assert rel.min() >= 0 and rel.max() < IDX_CAP
        idx_arr[t_of_e, p_of_e] = rel.astype(np.int16)
        s_arr[t_of_e, p_of_e, eslot] = enorm.astype(np.float16)

        # wrapped idx layout [16, tot*8], replicated to [128, tot*8]
        idx16 = np.zeros((16, p.tot * 8), dtype=np.int16)
        for ppart in range(128):
            idx16[ppart % 16, np.arange(p.tot) * 8 + ppart // 16] = idx_arr[:, ppart]
        idx128 = np.ascontiguousarray(np.tile(idx16, (8, 1)))

        # S tiles SBUF layout [128 slot, tot*WIN] fp16
        s_sb = np.ascontiguousarray(
            s_arr.transpose(1, 0, 2).reshape(128, p.tot * WIN)
        )
        p.per_core.append(dict(idx128=idx128, s_sb=s_sb))

    # ---- node-indexed arrays (host layouts) ----
    xf = np.asarray(x, dtype=np.float32)
    # layer-1 table: plain x, zero-padded to [npad, HID] fp16 (table-row order)
    tab1 = np.zeros((npad, HID), dtype=np.float16)
    tab1[trow, :F_IN] = xf.astype(np.float16)
    p.tab1 = tab1
    # per-core: x2own = x*dinv^2 [128, nsup, F_PAD] f32 and dinv2 [128, nsup]
    p.x2own = []
    p.dinv2 = []
    p.pmat = []
    for c in range(n_cores):
        lo = c * npc
        plid = prow[lo : lo + npc] - c * padc
        xo = np.zeros((128, nsup, F_PAD), dtype=np.float32)
        d2 = np.zeros((128, nsup), dtype=np.float32)
        dv2 = dinv[lo : lo + npc] * dinv[lo : lo + npc]
        xo[plid % 128, plid // 128, :F_IN] = xf[lo : lo + npc] * dv2[:, None]
        d2[plid % 128, plid // 128] = dv2
        p.x2own.append(np.ascontiguousarray(xo))
        p.dinv2.append(np.ascontiguousarray(d2))
        pm = np.zeros((128, nsup * n_graphs), dtype=np.float16)
        pm[plid % 128, (plid // 128) * n_graphs + batch[lo : lo + npc]] = 1.0
        p.pmat.append(np.ascontiguousarray(pm))

    p.identity = np.eye(128, dtype=np.float32)
    return p


def build_weight_arrays(p, W1, b1, W2, b2, W3, b3, Wl, bl):
    """Zero-pad / reshape weights (no arithmetic)."""
    w1p = np.zeros((F_PAD, HID), dtype=np.float32)
    w1p[:F_IN] = np.asarray(W1, dtype=np.float32)
    a = dict(
        w1=w1p,
        w2=np.asarray(W2, dtype=np.float32),
        w3=np.asarray(W3, dtype=np.float32),
        wl=np.asarray(Wl, dtype=np.float32),
        b1=np.asarray(b1, dtype=np.float32).reshape(HID, 1),
        b2=np.asarray(b2, dtype=np.float32).reshape(HID, 1),
        b3=np.asarray(b3, dtype=np.float32).reshape(HID, 1),
        blrep=np.ascontiguousarray(
            np.broadcast_to(np.asarray(bl, dtype=np.float32), (p.n_graphs, N_CLASS))
        ),
        invc=(1.0 / np.maximum(p.counts, 1.0)).reshape(p.n_graphs, 1),
    )
    return a


# ----------------------------------------------------------------------------
# Device program
# ----------------------------------------------------------------------------
def build_program(p, enable_asserts=False):
    import concourse.bass as bass
    import concourse.bacc as bacc
    import concourse.tile as tile
    import concourse.mybir as mybir

    dt = mybir.dt
    f32, f16, i16 = dt.float32, dt.float16, dt.int16
    Alu = mybir.AluOpType
    Act = mybir.ActivationFunctionType
    G = p.n_graphs
    rg = [list(range(p.n_cores))]

    nc = bacc.Bacc(
        "TRN2",
        target_bir_lowering=False,
        debug=False,
        enable_asserts=enable_asserts,
        num_devices=p.n_cores,
        num_swdge_queues=4,
    )

    # ---- DRAM tensors ----
    tab1_d = nc.dram_tensor("tab1", [p.npad, HID], f16, kind="ExternalInput")
    x2own_d = nc.dram_tensor("x2own", [128, p.nsup, F_PAD], f32, kind="ExternalInput")
    dinv2_d = nc.dram_tensor("dinv2", [128, p.nsup], f32, kind="ExternalInput")
    idx_d = nc.dram_tensor("idx", [128, p.tot * 8], i16, kind="ExternalInput")
    s_d = nc.dram_tensor("s_sb", [128, p.tot * WIN], f16, kind="ExternalInput")
    pmat_d = nc.dram_tensor("pmat", [128, p.nsup * G], f16, kind="ExternalInput")
    w1_d = nc.dram_tensor("w1", [F_PAD, HID], f32, kind="ExternalInput")
    w2_d = nc.dram_tensor("w2", [HID, HID], f32, kind="ExternalInput")
    w3_d = nc.dram_tensor("w3", [HID, HID], f32, kind="ExternalInput")
    wl_d = nc.dram_tensor("wl", [HID, N_CLASS], f32, kind="ExternalInput")
    b1_d = nc.dram_tensor("b1", [HID, 1], f32, kind="ExternalInput")
    b2_d = nc.dram_tensor("b2", [HID, 1], f32, kind="ExternalInput")
    b3_d = nc.dram_tensor("b3", [HID, 1], f32, kind="ExternalInput")
    invc_d = nc.dram_tensor("invc", [G, 1], f32, kind="ExternalInput")
    blrep_d = nc.dram_tensor("blrep", [G, N_CLASS], f32, kind="ExternalInput")
    ident_d = nc.dram_tensor("ident", [128, 128], f32, kind="ExternalInput")
    out_d = nc.dram_tensor("out", [G, N_CLASS], f32, kind="ExternalOutput")

    # Tables are AllGathered in two halves (a: supertiles < NSA, fired
    # mid-layer and fully hidden; b: at the layer end) and copied from the
    # Shared region into Internal lo/hi gather tables (gathers from Shared —
    # and from some ExternalInput placements — drain ~35% slower than from
    # compiler-placed Internal DRAM). lo = table rows [0, IDX_CAP); hi = rows
    # [npad-IDX_CAP, npad); separate tensors give sub-range dependencies.
    NA, NB = p.n_cores * p.na, p.n_cores * p.nb  # 24576, 25600
    HI0 = p.npad - IDX_CAP  # 17408
    nhi = p.npad - HI0  # 32768
    agin_a_d = nc.dram_tensor("agin_a", [p.na, HID], f16, kind="Internal")
    agin_b_d = nc.dram_tensor("agin_b", [p.nb, HID], f16, kind="Internal")
    tabs = {}
    for ln in (2, 3):
        tabs[ln] = dict(
            a=nc.dram_tensor(f"t{ln}a", [NA, HID], f16, kind="Internal",
                             addr_space="Shared"),
            b=nc.dram_tensor(f"t{ln}b", [NB, HID], f16, kind="Internal",
                             addr_space="Shared"),
            lo=nc.dram_tensor(f"t{ln}lo", [IDX_CAP, HID], f16, kind="Internal"),
            hi=nc.dram_tensor(f"t{ln}hi", [nhi, HID], f16, kind="Internal"),
        )
    tab1lo_d = nc.dram_tensor("tab1lo", [IDX_CAP, HID], f16, kind="Internal")
    tab1hi_d = nc.dram_tensor("tab1hi", [nhi, HID], f16, kind="Internal")
    arin_d = nc.dram_tensor("arin", [128, G], f32, kind="Internal")
    arout_d = nc.dram_tensor(
        "arout", [128, G], f32, kind="Internal", addr_space="Shared"
    )
    warm_in_d = nc.dram_tensor("warm_in", [128, 8], f32, kind="Internal")
    warm_out_d = nc.dram_tensor(
        "warm_out", [128, 64], f32, kind="Internal", addr_space="Shared"
    )

    with tile.TileContext(nc) as tc:
        with (
            tc.tile_pool(name="const", bufs=1) as cpool,
            tc.tile_pool(name="gath", bufs=GATH_BUFS) as gpool,
            tc.tile_pool(name="stage", bufs=3) as stpool,
            tc.tile_pool(name="psagg", bufs=GROUP_SUPS + 1, space="PSUM") as psagg,
            tc.tile_pool(name="psstg", bufs=2, space="PSUM") as psstg,
            tc.tile_pool(name="psacc", bufs=1, space="PSUM") as psacc,
        ):
            # ---- persistent SBUF tiles ----
            # HWDGE executes in emission order: put the lo-table copy and the
            # gather indices first so layer-1 gathers start ~50us earlier; the
            # hi copy / S tiles drain behind them.
            nc.sync.dma_start(tab1lo_d[:, :], tab1_d[0:IDX_CAP, :])
            idx_sb = cpool.tile([128, p.tot * 8], i16, tag="idx")
            nc.sync.dma_start(idx_sb[:, :], idx_d[:, :])
            nc.sync.dma_start(tab1hi_d[:, :], tab1_d[HI0 : p.npad, :])
            s_sb = cpool.tile([128, p.tot * WIN], f16, tag="s_sb")
            nc.sync.dma_start(s_sb[:, :], s_d[:, :])
            ident = cpool.tile([128, 128], f32, tag="ident")
            nc.sync.dma_start(ident[:, :], ident_d[:, :])
            w1 = cpool.tile([F_PAD, HID], f32, tag="w1")
            nc.sync.dma_start(w1[:, :], w1_d[:, :])
            w2 = cpool.tile([HID, HID], f32, tag="w2")
            nc.sync.dma_start(w2[:, :], w2_d[:, :])
            w3 = cpool.tile([HID, HID], f32, tag="w3")
            nc.sync.dma_start(w3[:, :], w3_d[:, :])
            wl = cpool.tile([HID, N_CLASS], f32, tag="wl")
            nc.sync.dma_start(wl[:, :], wl_d[:, :])
            b1 = cpool.tile([HID, 1], f32, tag="b1")
            nc.sync.dma_start(b1[:, :], b1_d[:, :])
            b2 = cpool.tile([HID, 1], f32, tag="b2")
            nc.sync.dma_start(b2[:, :], b2_d[:, :])
            b3 = cpool.tile([HID, 1], f32, tag="b3")
            nc.sync.dma_start(b3[:, :], b3_d[:, :])
            invc = cpool.tile([G, 1], f32, tag="invc")
            nc.sync.dma_start(invc[:, :], invc_d[:, :])
            blrep = cpool.tile([G, N_CLASS], f32, tag="blrep")
            nc.sync.dma_start(blrep[:, :], blrep_d[:, :])
            pmat = cpool.tile([128, p.nsup * G], f16, tag="pmat")
            nc.sync.dma_start(pmat[:, :], pmat_d[:, :])
            dinv2 = cpool.tile([128, p.nsup], f32, tag="dinv2")
            nc.sync.dma_start(dinv2[:, :], dinv2_d[:, :])
            x2own = cpool.tile([128, p.nsup, F_PAD], f32, tag="x2own")
            nc.sync.dma_start(x2own[:, :, :], x2own_d[:, :, :])
            hown2a = cpool.tile([128, p.nsup * HID], f16, tag="hown2a")
            hown2b = cpool.tile([128, p.nsup * HID], f16, tag="hown2b")

            # warm-up collective: absorbs first-collective latency + aligns
            # the cores before the timed layers
            nc.sync.dma_start(warm_in_d[:, :], ident[:, 0:8])
            nc.gpsimd.collective_compute(
                "AllGather",
                Alu.bypass,
                replica_groups=rg,
                ins=[warm_in_d[:, :]],
                outs=[warm_out_d[:, :]],
            )

            # ---- persistent PSUM tiles ----
            pacc = psacc.tile([128, G], f32, tag="pacc")

            # ---- layers ----
            layers = [
                (0, tab1lo_d, tab1hi_d, w1, b1, True, tabs[2], None, hown2a),
                (1, tabs[2]["lo"], tabs[2]["hi"], w2, b2, True, tabs[3], hown2a, hown2b),
                (2, tabs[3]["lo"], tabs[3]["hi"], w3, b3, False, None, hown2b, None),
            ]
            for li, tlo_d, thi_d, w_sb, b_sb, relu, tnext, hin, hout in layers:
                fdim = F_PAD if li == 0 else HID
                lo_view = tlo_d[:, :]
                hi_view = thi_d[:, :]
                for s0, nsg, chunk_ids, _tr in p.groups:
                    aggs = [
                        psagg.tile([128, HID], f32, tag="agg", name="agg")
                        for _ in range(nsg)
                    ]
                    for ci in chunk_ids:
                        t0, ntl, cls = p.chunks[ci]
                        gt = gpool.tile([128, CHUNK_MAX, HID], f16, tag="gath")
                        view = hi_view if cls == 1 else lo_view
                        nc.gpsimd.dma_gather(
                            gt[:, :ntl, :],
                            view,
                            idx_sb[:, t0 * 8 : (t0 + ntl) * 8],
                            ntl * 128,
                            ntl * 128,
                            HID,
                            elem_step=HID,
                            single_packet=SINGLE_PACKET,
                            queue_num=p.chunk_queue[ci],
                        )
                        for j in range(ntl):
                            t = t0 + j
                            w, _cls = p.tiles[t]
                            sj = (w // 4) - s0
                            pb = (w % 4) * WIN
                            nc.tensor.matmul(
                                aggs[sj][pb : pb + WIN, 0:fdim],
                                s_sb[:, t * WIN : (t + 1) * WIN],
                                gt[:, j, 0:fdim],
                                start=bool(p.first_of_win[t]),
                                stop=bool(p.last_of_win[t]),
                                tile_position=(0, pb),
                                skip_group_check=True,
                            )
                    for sj in range(nsg):
                        s = s0 + sj
                        psum_agg = aggs[sj][:, 0:fdim]
                        stg = psstg.tile([128, 384], f32, tag="stg")
                        uTps = stg[:, 0:128]
                        hTps = stg[:, 128:256]
                        hbps = stg[:, 256:384]
                        u = stpool.tile([128, HID], f32, tag="u")
                        if li == 0:
                            nc.vector.tensor_tensor(
                                u[:, 0:F_PAD],
                                psum_agg,
                                x2own[:, s, :],
                                Alu.add,
                            )
                        else:
                            nc.vector.tensor_tensor(
                                u[:, :],
                                psum_agg,
                                hin[:, s * HID : (s + 1) * HID],
                                Alu.add,
                            )
                        nc.tensor.transpose(
                            uTps[0:fdim, :], u[:, 0:fdim], ident[:, :]
                        )
                        uT = stpool.tile([128, 128], f32, tag="uTs")
                        nc.vector.tensor_copy(uT[0:fdim, :], uTps[0:fdim, :])
                        nc.tensor.matmul(
                            hTps,
                            w_sb[0:fdim, :],
                            uT[0:fdim, :],
                            start=True,
                            stop=True,
                        )
                        hT = stpool.tile([128, 128], f32, tag="hTs")
                        if relu:
                            nc.scalar.activation(
                                hT[:, :],
                                hTps,
                                Act.Relu,
                                bias=b_sb[:, 0:1],
                            )
                        else:
                            nc.vector.tensor_scalar(
                                hT[:, :], hTps, b_sb[:, 0:1], None, Alu.add
                            )
                        nc.tensor.transpose(hbps, hT[:, :], ident[:, :])
                        if li < 2:
                            hf = stpool.tile([128, 128], f16, tag="hf")
                            nc.vector.tensor_copy(hf[:, :], hbps)
                            nc.vector.tensor_scalar(
                                hout[:, s * HID : (s + 1) * HID],
                                hbps,
                                dinv2[:, s : s + 1],
                                None,
                                Alu.mult,
                            )
                            if s < NSA:
                                ag_dst = agin_a_d[:, :].rearrange(
                                    "(t q) f -> q t f", q=128
                                )[:, s, :]
                            else:
                                ag_dst = agin_b_d[:, :].rearrange(
                                    "(t q) f -> q t f", q=128
                                )[:, s - NSA, :]
                            nc.sync.dma_start(ag_dst, hf[:, :])
                        else:
                            h3 = stpool.tile([128, 128], f16, tag="hf")
                            nc.vector.tensor_copy(h3[:, :], hbps)
                            nc.tensor.matmul(
                                pacc[:, 0:G],
                                h3[:, :],
                                pmat[:, s * G : (s + 1) * G],
                                start=(s == 0),
                                stop=(s == p.nsup - 1),
                                skip_group_check=True,
                            )
                    # emit the early-half AllGather a couple of groups after
                    # its inputs complete so the trigger's sem waits are
                    # already satisfied when the gpsimd queue reaches it
                    if tnext is not None and s0 + nsg == NSA + 2 * GROUP_SUPS:
                        nc.gpsimd.collective_compute(
                            "AllGather",
                            Alu.bypass,
                            replica_groups=rg,
                            ins=[agin_a_d[:, :]],
                            outs=[tnext["a"][:, :]],
                        )
                        nc.sync.dma_start(
                            tnext["lo"][0:NA, :], tnext["a"][:, :]
                        )
                        nc.sync.dma_start(
                            tnext["hi"][0 : NA - HI0, :], tnext["a"][HI0:NA, :]
                        )
                if tnext is not None:
                    nc.gpsimd.collective_compute(
                        "AllGather",
                        Alu.bypass,
                        replica_groups=rg,
                        ins=[agin_b_d[:, :]],
                        outs=[tnext["b"][:, :]],
                    )
                    nc.sync.dma_start(
                        tnext["lo"][NA:IDX_CAP, :], tnext["b"][0 : IDX_CAP - NA, :]
                    )
                    nc.sync.dma_start(
                        tnext["hi"][NA - HI0 : nhi, :], tnext["b"][:, :]
                    )

            # ---- pooling finalize + classifier ----
            pooledT = stpool.tile([128, G], f32, tag="pool")
            nc.vector.tensor_copy(pooledT[:, :], pacc[:, 0:G])
            nc.sync.dma_start(arin_d[:, :], pooledT[:, :])
            nc.gpsimd.collective_compute(
                "AllReduce",
                Alu.add,
                replica_groups=rg,
                ins=[arin_d[:, :]],
                outs=[arout_d[:, :]],
            )
            pooled2 = stpool.tile([128, G], f32, tag="pool")
            nc.sync.dma_start(pooled2[:, :], arout_d[:, :])
            lgps = psstg.tile([128, 384], f32, tag="stg")
            nc.tensor.matmul(
                lgps[0:G, 0:N_CLASS], pooled2[:, :], wl[:, :], start=True, stop=True
            )
            outt = stpool.tile([G, N_CLASS], f32, tag="out")
            nc.vector.scalar_tensor_tensor(
                outt[:, :],
                lgps[0:G, 0:N_CLASS],
                invc[:, 0:1],
                blrep[:, :],
                Alu.mult,
                Alu.add,
            )
            nc.sync.dma_start(out_d[:, :], outt[:, :])

    nc.compile()
    return nc


def make_in_maps(p, wa):
    maps = []
    for c in range(p.n_cores):
        pc = p.per_core[c]
        maps.append(
            dict(
                tab1=p.tab1,
                x2own=p.x2own[c],
                dinv2=p.dinv2[c],
                idx=pc["idx128"],
                s_sb=pc["s_sb"],
                pmat=p.pmat[c],
                w1=wa["w1"],
                w2=wa["w2"],
                w3=wa["w3"],
                wl=wa["wl"],
                b1=wa["b1"],
                b2=wa["b2"],
                b3=wa["b3"],
                invc=wa["invc"],
                blrep=wa["blrep"],
                ident=p.identity,
            )
        )
    return maps


_CACHE = {}


def kernel(x, edge_index, edge_attr, batch, W1, b1, W2, b2, W3, b3, Wl, bl):
    x = np.asarray(x)
    p = build_plan(x, np.asarray(edge_index), np.asarray(edge_attr), np.asarray(batch))
    wa = build_weight_arrays(p, W1, b1, W2, b2, W3, b3, Wl, bl)
    key = (p.n, p.tot)
    if key not in _CACHE:
        _CACHE[key] = build_program(p)
    nc = _CACHE[key]
    from concourse.bass_utils import run_bass_kernel_spmd

    res = run_bass_kernel_spmd(nc, make_in_maps(p, wa), core_ids=list(range(p.n_cores)))
    return np.asarray(res.results[0]["out"], dtype=np.float32)
